# revision 31
# baseline (speedup 1.0000x reference)
"""Trainium2 Bass kernel for nn_BiaffineScoreLayer (segment_reduce).

Strategy: data-parallel over batch B=8 -> one example per NeuronCore.
Per core, everything is computed on-device:
  - gated fusion via tanh identity: fg = (s + tanh(pre/2)*d)/2 with s=f+c, d=f-c
    (the factor 1/2 is folded into prescaled U on the host)
  - biaffine scores via PE matmuls + fused DVE multiply-reduce
  - argmax(==2) indicator, strict prefix-sum via triangular matmuls,
    L x L equality mask in fp16 on DVE
  - mid-feature linears (al1/al2/al3) via K=7 matmuls with the row mask
    folded into the PSUM->SBUF copy scale

Host side only reshapes/shards inputs (layout marshalling) and gathers
outputs; all FLOPs of the reference live on the device.
"""

import sys

if "/opt/trn_rl_repo" not in sys.path:
    sys.path.insert(0, "/opt/trn_rl_repo")

import numpy as np

B, L, D, C = 8, 2048, 512, 3
P = 128
NT = L // P          # 16 token tiles
NCH = L // 512       # 4 token chunks
NK = D // P          # 4 feature tiles

_CACHE = {}


def _build_nc():
    import os
    import concourse.bacc as bacc
    import concourse.mybir as mybir
    import concourse.tile as tile

    stage = int(os.environ.get("BK_STAGE", "9"))

    dt = mybir.dt
    f32, f16, u8 = dt.float32, dt.float16, dt.uint8
    AF = mybir.ActivationFunctionType
    OP = mybir.AluOpType

    nc = bacc.Bacc("TRN2", target_bir_lowering=False, debug=False, num_devices=8)

    # ---- DRAM inputs (per-core, host-marshalled layouts) ----
    # activations: [chunk, p, dtile, col] so each chunk is one contiguous DMA
    sft = nc.dram_tensor("sft", [NCH, P, NK, 512], f32, kind="ExternalInput")
    dft = nc.dram_tensor("dft", [NCH, P, NK, 512], f32, kind="ExternalInput")
    sbt = nc.dram_tensor("sbt", [NCH, P, NK, 512], f32, kind="ExternalInput")  # shifted +1 token
    sbn = nc.dram_tensor("sbn", [L, D], f32, kind="ExternalInput")
    dbn = nc.dram_tensor("dbn", [L, D], f32, kind="ExternalInput")
    wgf = nc.dram_tensor("wgf", [P, NK, 512], f32, kind="ExternalInput")
    wgb = nc.dram_tensor("wgb", [P, NK, 512], f32, kind="ExternalInput")
    bgfh = nc.dram_tensor("bgfh", [P, NK], f32, kind="ExternalInput")   # 0.5*bgf tiled
    bgbr = nc.dram_tensor("bgbr", [1, 512], f32, kind="ExternalInput")
    ua = nc.dram_tensor("ua", [P, C * NK, 512], f32, kind="ExternalInput")  # 0.25*U[:, :D, :D]
    uh = nc.dram_tensor("uh", [P, NK, C], f32, kind="ExternalInput")        # 0.5*U[:, :D, D]
    vrow = nc.dram_tensor("vrow", [1, C * 512], f32, kind="ExternalInput")  # 0.5*U[:, D, :D] flat
    sval = nc.dram_tensor("sval", [1, C], f32, kind="ExternalInput")        # U[:, D, D]
    wea = nc.dram_tensor("wea", [4, C, 512], f32, kind="ExternalInput")     # [We_i rows 0-2; be_i]
    web = nc.dram_tensor("web", [3, C, 512], f32, kind="ExternalInput")     # We_i rows 3-5
    maskbig = nc.dram_tensor("maskbig", [P, NT], f32, kind="ExternalInput")  # 4096*mask (X-layout)
    invmask = nc.dram_tensor("invmask", [P, NT], f32, kind="ExternalInput")  # 1-mask (X-layout)

    # ---- DRAM outputs ----
    masks_o = nc.dram_tensor("masks_o", [L, L], u8, kind="ExternalOutput")
    aswt_o = nc.dram_tensor("aswt_o", [C, L + 1], f32, kind="ExternalOutput")
    al_o = [
        nc.dram_tensor(f"al{i}_o", [L, D], f32, kind="ExternalOutput") for i in range(3)
    ]

    # ---- NEFF-embedded constants ----
    id_np = np.eye(P, dtype=np.float32)
    su_np = np.triu(np.ones((P, P), np.float32), 1)  # [q, p] = 1 iff q < p
    on_np = np.ones((P, P), np.float32)
    id_dr = nc.inline_tensor(id_np, "id128")
    su_dr = nc.inline_tensor(su_np, "su128")
    on_dr = nc.inline_tensor(on_np, "ones128")
    # row 0 = ones (DMA-source for bufA ones row)
    misc_np = np.zeros((1, L + 1), np.float32)
    misc_np[0] = 1.0
    misc_dr = nc.inline_tensor(misc_np, "misc4")

    with tile.TileContext(nc) as tc:
        with (
            tc.tile_pool(name="per", bufs=1) as per,
            tc.tile_pool(name="pin", bufs=2) as pin,
            tc.tile_pool(name="pbn", bufs=2) as pbn,
            tc.tile_pool(name="pth", bufs=2) as pth,
            tc.tile_pool(name="pm", bufs=1) as pm,
            tc.tile_pool(name="pbgs", bufs=2) as pbgs,
            tc.tile_pool(name="pscr", bufs=1) as pscr,
            tc.tile_pool(name="pal", bufs=2) as pal,
            tc.tile_pool(name="pmask", bufs=2) as pmask,
            tc.tile_pool(name="pasw", bufs=2) as pasw,
            tc.tile_pool(name="pp_pre", bufs=1, space="PSUM") as pp_pre,
            tc.tile_pool(name="pp_hh", bufs=3, space="PSUM") as pp_hh,
            tc.tile_pool(name="pp_up", bufs=1, space="PSUM") as pp_up,
            tc.tile_pool(name="pp_al", bufs=1, space="PSUM") as pp_al,
            tc.tile_pool(name="pp_tiny", bufs=1, space="PSUM") as pp_tiny,
            tc.tile_pool(name="pp_qq", bufs=1, space="PSUM") as pp_qq,
        ):
            # persistent SBUF
            wgf_sb = per.tile([P, NK, 512], f32, name="wgf_sb")
            wgb_sb = per.tile([P, NK, 512], f32, name="wgb_sb")
            bgfh_sb = per.tile([P, NK], f32, name="bgfh_sb")
            bgbr_sb = per.tile([1, 512], f32, name="bgbr_sb")
            ua_sb = per.tile([P, C * NK, 512], f32, name="ua_sb")
            uh_sb = per.tile([P, NK, C], f32, name="uh_sb")
            vrow_sb = per.tile([1, C * 512], f32, name="vrow_sb")
            sval_sb = per.tile([1, C], f32, name="sval_sb")
            wea_sb = per.tile([4, C, 512], f32, name="wea_sb")
            web_sb = per.tile([3, C, 512], f32, name="web_sb")
            maskbig_sb = per.tile([P, NT], f32, name="maskbig_sb")
            invmask_sb = per.tile([P, NT], f32, name="invmask_sb")
            id_sb = per.tile([P, P], f32, name="id_sb")
            su_sb = per.tile([P, P], f32, name="su_sb")
            ones_sb = per.tile([P, P], f32, name="ones_sb")
            fgs_sb = per.tile([P, NK, L], f32, name="fgs_sb")
            bufa = per.tile([4, L + 1], f32, name="bufa")
            x2 = per.tile([P, NT], f32, name="x2")
            qsb = per.tile([P, NT], f32, name="qsb")
            segm16 = per.tile([P, NT], f32, name="segm16")
            qt_sb = per.tile([NT, P], f32, name="qt_sb")
            qflat = per.tile([1, L], f32, name="qflat")
            qb16 = per.tile([P, L], f16, name="qb16")
            t_sb = per.tile([1, NT], f32, name="t_sb")
            tt_sb = per.tile([NT, 1], f32, name="tt_sb")
            carry_sb = per.tile([1, NT], f32, name="carry_sb")

            for t, dr in [
                (wgf_sb, wgf), (wgb_sb, wgb), (bgfh_sb, bgfh), (bgbr_sb, bgbr),
                (ua_sb, ua), (uh_sb, uh), (vrow_sb, vrow), (sval_sb, sval),
                (wea_sb, wea), (web_sb, web), (maskbig_sb, maskbig), (invmask_sb, invmask),
                (id_sb, id_dr), (su_sb, su_dr), (ones_sb, on_dr),
            ]:
                nc.sync.dma_start(t[:], dr.ap())

            nc.vector.memset(x2[:], 0.0)
            nc.vector.memset(bufa[0:3, :], 0.0)
            misc_sb = per.tile([1, L + 1], f32, name="misc_sb")
            nc.sync.dma_start(misc_sb[:], misc_dr.ap())
            nc.sync.dma_start(bufa[3:4, :], misc_sb[0:1, :])

            for ch in range(NCH):
                c0 = ch * 512
                sft_t = pin.tile([P, NK, 512], f32, name="sft_t", tag="sft")
                dft_t = pin.tile([P, NK, 512], f32, name="dft_t", tag="dft")
                sbt_t = pin.tile([P, NK, 512], f32, name="sbt_t", tag="sbt")
                nc.sync.dma_start(sft_t[:], sft.ap()[ch])
                nc.sync.dma_start(dft_t[:], dft.ap()[ch])
                nc.sync.dma_start(sbt_t[:], sbt.ap()[ch])

                # ---- f-side gating (transposed space) ----
                for j in range(NK):
                    pre = pp_pre.tile([P, 512], f32, name="pre", tag="pre")
                    for q in range(NK):
                        nc.tensor.matmul(
                            pre[:], wgf_sb[:, q, j * P : (j + 1) * P], sft_t[:, q, :],
                            start=(q == 0), stop=(q == NK - 1),
                        )
                    th = pth.tile([P, 512], f32, name="th", tag="th")
                    nc.scalar.activation(
                        th[:], pre[:], AF.Tanh, bias=bgfh_sb[:, j : j + 1], scale=0.5
                    )
                    m = pm.tile([P, 512], f32, name="m", tag="m")
                    nc.vector.scalar_tensor_tensor(
                        m[:], th[:], 0.0, dft_t[:, j, :], OP.bypass, OP.mult
                    )
                    nc.vector.scalar_tensor_tensor(
                        fgs_sb[:, j, c0 : c0 + 512], m[:], 0.0, sft_t[:, j, :],
                        OP.bypass, OP.add,
                    )

                if stage < 2:
                    continue
                for jj in range(4):
                    k = ch * 4 + jj
                    t0 = k * P
                    M = P - 1 if k == NT - 1 else P

                    # ---- b-side gating (natural space, shifted +1 token) ----
                    sb_t = pbn.tile([P, 512], f32, name="sb_t", tag="sbn")
                    db_t = pbn.tile([P, 512], f32, name="db_t", tag="dbn")
                    nc.sync.dma_start(sb_t[:M], sbn.ap()[t0 + 1 : t0 + 1 + M, :])
                    nc.sync.dma_start(db_t[:M], dbn.ap()[t0 + 1 : t0 + 1 + M, :])
                    preb = pp_pre.tile([P, 512], f32, name="preb", tag="pre")
                    for q in range(NK):
                        nc.tensor.matmul(
                            preb[:M], sbt_t[:, q, jj * P : jj * P + M], wgb_sb[:, q, :],
                            start=(q == 0), stop=False,
                        )
                    nc.tensor.matmul(
                        preb[:M], ones_sb[0:1, 0:M], bgbr_sb[:], start=False, stop=True
                    )
                    thb = pth.tile([P, 512], f32, name="thb", tag="th")
                    nc.scalar.activation(thb[:M], preb[:M], AF.Tanh, scale=0.5)
                    mb = pm.tile([P, 512], f32, name="mb", tag="m")
                    nc.vector.scalar_tensor_tensor(
                        mb[:M], thb[:M], 0.0, db_t[:M], OP.bypass, OP.mult
                    )
                    bgs = pbgs.tile([P, 512], f32, name="bgs", tag="bgs")
                    nc.vector.scalar_tensor_tensor(
                        bgs[:M], mb[:M], 0.0, sb_t[:M], OP.bypass, OP.add
                    )

                    if stage < 3:
                        continue
                    # ---- biaffine: H_c = fgs @ (U_c/4) + v_c/2 ; u-terms ----
                    hhs = []
                    for cc in range(C):
                        hh = pp_hh.tile([P, 512], f32, name="hh", tag="hh")
                        hhs.append(hh)
                        for i in range(NK):
                            nc.tensor.matmul(
                                hh[:M, :],
                                fgs_sb[:, i, t0 : t0 + M], ua_sb[:, cc * NK + i, :],
                                start=(i == 0), stop=False,
                            )
                        nc.tensor.matmul(
                            hh[:M, :],
                            ones_sb[0:1, 0:M], vrow_sb[0:1, cc * 512 : (cc + 1) * 512],
                            start=False, stop=True,
                        )
                    # u-term: [M, 3] = fgs @ (u_c/2) + s_c in one N=3 group
                    u_ps = pp_up.tile([P, 4], f32, name="u_ps", tag="up")
                    for i in range(NK):
                        nc.tensor.matmul(
                            u_ps[:M, 0:C],
                            fgs_sb[:, i, t0 : t0 + M], uh_sb[:, i, 0:C],
                            start=(i == 0), stop=False,
                        )
                    nc.tensor.matmul(
                        u_ps[:M, 0:C],
                        ones_sb[0:1, 0:M], sval_sb[0:1, 0:C],
                        start=False, stop=True,
                    )

                    asw_nat = pasw.tile([P, 4], f32, name="asw_nat", tag="asw")
                    acc3 = pasw.tile([P, 4], f32, name="acc3", tag="acc")
                    for cc in range(C):
                        scr = pscr.tile([P, 512], f32, name="scr", tag="scr")
                        nc.vector.scalar_tensor_tensor(
                            scr[:M], hhs[cc][:M, :], 0.0, bgs[:M],
                            OP.bypass, OP.mult, accum_out=acc3[:M, cc : cc + 1],
                        )
                    nc.vector.tensor_add(asw_nat[:M, 0:C], acc3[:M, 0:C], u_ps[:M, 0:C])
                    # is2 = asw2 > max(asw0, asw1)  (strict argmax==2)
                    nc.vector.tensor_max(
                        asw_nat[:M, 3:4], asw_nat[:M, 0:1], asw_nat[:M, 1:2]
                    )
                    nc.vector.tensor_tensor(
                        x2[:M, k : k + 1], asw_nat[:M, 2:3], asw_nat[:M, 3:4], OP.is_gt
                    )

                    # aswT into bufa rows 0-2 at columns t+1 (col 0 is zero pad)
                    tp = pp_tiny.tile([NT, P], f32, name="tp", tag="tp")
                    nc.tensor.transpose(
                        tp[0:3, 0:M], asw_nat[0:M, 0:3], id_sb[0:M, 0:M]
                    )
                    nc.scalar.activation(
                        bufa[0:3, t0 + 1 : t0 + 1 + M], tp[0:3, 0:M], AF.Copy
                    )

                    # ---- al1/al2/al3: asw1-part (+bias) and asw2-part matmuls ----
                    if stage < 4:
                        continue
                    for i in range(3):
                        alp = pp_al.tile([P, 512], f32, name="alp", tag="alp")
                        nc.tensor.matmul(
                            alp[:], bufa[0:4, t0 + 1 : t0 + 1 + P], wea_sb[:, i, :],
                            start=True, stop=False,
                        )
                        nc.tensor.matmul(
                            alp[:], bufa[0:3, t0 : t0 + P], web_sb[:, i, :],
                            start=False, stop=True,
                        )
                        al_sb = pal.tile([P, 512], f32, name="al_sb", tag="al")
                        nc.scalar.activation(
                            al_sb[:], alp[:], AF.Copy, scale=invmask_sb[:, k : k + 1]
                        )
                        nc.sync.dma_start(al_o[i].ap()[t0 : t0 + P, :], al_sb[:])

            if stage < 2:
                nc.sync.dma_start(al_o[0].ap()[0:P, :], fgs_sb[:, 0, 0:512])
            if stage >= 3:
                nc.sync.dma_start(aswt_o.ap()[:, :], bufa[0:3, :])
            if stage >= 5:
                # ---- strict prefix sum Q (two-level) ----
                # within-chunk strict cumsum: Y = SU^T X
                q_ps = pp_qq.tile([P, NT], f32, name="q_ps", tag="qq")
                nc.tensor.matmul(
                    q_ps[:, 0:NT], su_sb[:], x2[:, 0:NT],
                    start=True, stop=False, skip_group_check=True,
                )
                # chunk sums T[k] = colsum(X[:, k])
                t_ps = pp_tiny.tile([1, NT], f32, name="t_ps", tag="tp")
                nc.tensor.matmul(
                    t_ps[0:1, 0:NT], ones_sb[:, 0:1], x2[:, 0:NT],
                    start=True, stop=True, skip_group_check=True,
                )
                nc.vector.tensor_copy(t_sb[:], t_ps[0:1, :])
                # transpose T to [NT, 1]
                tt_ps = pp_tiny.tile([NT, 1], f32, name="tt_ps", tag="tp")
                nc.tensor.transpose(tt_ps[0:NT, 0:1], t_sb[0:1, 0:NT], id_sb[0:1, 0:1])
                nc.vector.tensor_copy(tt_sb[:], tt_ps[0:NT, :])
                # strict cumsum of chunk sums: carry[k] = sum_{q<k} T[q]
                carry_ps = pp_tiny.tile([1, NT], f32, name="carry_ps", tag="tp")
                nc.tensor.matmul(
                    carry_ps[0:1, 0:NT], tt_sb[0:NT, 0:1], su_sb[0:NT, 0:NT],
                    start=True, stop=True, skip_group_check=True,
                )
                nc.vector.tensor_copy(carry_sb[:], carry_ps[0:1, :])
                # broadcast-accumulate carry into Q
                nc.tensor.matmul(
                    q_ps[:, 0:NT], ones_sb[0:1, 0:P], carry_sb[0:1, 0:NT],
                    start=False, stop=True, skip_group_check=True,
                )
                nc.vector.tensor_copy(qsb[:], q_ps[:])
                # row scalars with row-mask folded in: Q - 4096*mask
                nc.vector.scalar_tensor_tensor(
                    segm16[:], qsb[:], 0.0, maskbig_sb[:], OP.bypass, OP.subtract
                )
                # flatten Q to [1, L]: PE transpose + sbuf->sbuf DMA
                qt_ps = pp_tiny.tile([NT, P], f32, name="qt_ps", tag="tp")
                nc.tensor.transpose(qt_ps[0:NT, 0:P], qsb[:, 0:NT], id_sb[:, :])
                nc.scalar.activation(qt_sb[:], qt_ps[0:NT, :], AF.Copy)
                nc.sync.dma_start(qflat[0:1, :], qt_sb[:])
                # broadcast Q over partitions (fp16)
                for cch in range(NCH):
                    bp = pp_tiny.tile([P, 512], f32, name="bp", tag="tp")
                    nc.tensor.matmul(
                        bp[:], ones_sb[0:1, 0:P], qflat[0:1, cch * 512 : (cch + 1) * 512],
                        start=True, stop=True,
                    )
                    nc.vector.tensor_copy(qb16[:, cch * 512 : (cch + 1) * 512], bp[:])
                # L x L equality mask
                for k in range(NT):
                    mt = pmask.tile([P, L], u8, name="mt", tag="mask")
                    nc.vector.tensor_scalar(
                        mt[:], qb16[:], segm16[:, k : k + 1], None, OP.is_equal
                    )
                    nc.sync.dma_start(masks_o.ap()[k * P : (k + 1) * P, :], mt[:])

    nc.compile()
    return nc


def _prep_core_inputs(bi, f, c, b, mask, Wgf, bgf, Wgb, bgb, U, WE):
    fb, cb, bb = f[bi], c[bi], b[bi]
    s_f = fb + cb
    d_f = fb - cb
    s_b = bb + cb
    d_b = bb - cb

    def chunks_T(x, shift):
        # x: [L, D] -> transposed chunked layout [NCH, P, NK, 512] over tokens,
        # token columns optionally shifted by +1 (padded with zero)
        xt = x.T  # [D, L]
        if shift:
            xt = np.concatenate([xt[:, 1:], np.zeros((D, 1), np.float32)], axis=1)
        out = np.empty((NCH, P, NK, 512), np.float32)
        for ch in range(NCH):
            blk = xt[:, ch * 512 : (ch + 1) * 512]  # [D, 512]
            out[ch] = blk.reshape(NK, P, 512).transpose(1, 0, 2)
        return out

    mk = mask[bi, 0].astype(np.float32)  # [L]
    mk_x = mk.reshape(NT, P).T.copy()    # [P, NT]

    return {
        "sft": chunks_T(s_f, False),
        "dft": chunks_T(d_f, False),
        "sbt": chunks_T(s_b, True),
        "sbn": np.ascontiguousarray(s_b),
        "dbn": np.ascontiguousarray(d_b),
        "wgf": Wgf.reshape(NK, P, 512).transpose(1, 0, 2).copy(),
        "wgb": Wgb.reshape(NK, P, 512).transpose(1, 0, 2).copy(),
        "bgfh": (0.5 * bgf).reshape(NK, P).T.copy(),
        "bgbr": bgb.reshape(1, 512).copy(),
        "ua": (0.25 * U[:, :D, :D]).reshape(C, NK, P, 512).transpose(2, 0, 1, 3).reshape(P, C * NK, 512).copy(),
        "uh": (0.5 * U[:, :D, D]).T.reshape(NK, P, C).transpose(1, 0, 2).copy(),
        "vrow": (0.5 * U[:, D, :D]).reshape(1, C * 512).copy(),
        "sval": U[:, D, D].reshape(1, C).copy(),
        "wea": WE[0],
        "web": WE[1],
        "maskbig": 4096.0 * mk_x,
        "invmask": 1.0 - mk_x,
    }


def _pack_we(We1, be1, We2, be2, We3, be3):
    # al1 <- (We1, be1); al2 <- (We3, be3); al3 <- (We2, be2)  (reference quirk)
    # wea = [We rows 0-2; be], web = We rows 3-5
    pairs = [(We1, be1), (We3, be3), (We2, be2)]
    wea = np.stack(
        [
            np.concatenate([np.asarray(w, np.float32)[0:3], np.asarray(bias, np.float32)[None, :]], 0)
            for w, bias in pairs
        ],
        axis=1,
    ).copy()  # [4, 3, 512]
    web = np.stack(
        [np.asarray(w, np.float32)[3:6] for w, _ in pairs], axis=1
    ).copy()  # [3, 3, 512]
    return (wea, web)


def kernel(f, c, b, mask, Wgf, bgf, Wgb, bgb, U, We1, be1, We2, be2, We3, be3):
    from concourse import bass_utils

    if "nc" not in _CACHE:
        _CACHE["nc"] = _build_nc()
    nc = _CACHE["nc"]

    f = np.asarray(f, np.float32)
    c = np.asarray(c, np.float32)
    b = np.asarray(b, np.float32)
    mask = np.asarray(mask)
    U = np.asarray(U, np.float32)
    WE = _pack_we(We1, be1, We2, be2, We3, be3)

    in_maps = [
        _prep_core_inputs(
            bi, f, c, b, mask,
            np.asarray(Wgf, np.float32), np.asarray(bgf, np.float32),
            np.asarray(Wgb, np.float32), np.asarray(bgb, np.float32),
            U, WE,
        )
        for bi in range(B)
    ]

    r = bass_utils.run_bass_kernel_spmd(nc, in_maps, core_ids=list(range(B)))

    asw = np.stack([r.results[bi]["aswt_o"][:, 1:L].T for bi in range(B)])
    masks = np.stack([r.results[bi]["masks_o"].astype(bool) for bi in range(B)])
    al1 = np.stack([r.results[bi]["al0_o"] for bi in range(B)])
    al2 = np.stack([r.results[bi]["al1_o"] for bi in range(B)])
    al3 = np.stack([r.results[bi]["al2_o"] for bi in range(B)])
    return asw, masks, al1, al2, al3


# revision 37
# speedup vs baseline: 1.0083x; 1.0083x over previous
"""Trainium2 Bass kernel for nn_BiaffineScoreLayer (segment_reduce).

Strategy: data-parallel over batch B=8 -> one example per NeuronCore.
Per core, everything is computed on-device:
  - gated fusion via tanh identity: fg = (s + tanh(pre/2)*d)/2 with s=f+c, d=f-c
    (the factor 1/2 is folded into prescaled U on the host)
  - biaffine scores via PE matmuls + fused DVE multiply-reduce
  - argmax(==2) indicator, strict prefix-sum via triangular matmuls,
    L x L equality mask in fp16 on DVE
  - mid-feature linears (al1/al2/al3) via K=7 matmuls with the row mask
    folded into the PSUM->SBUF copy scale

Host side only reshapes/shards inputs (layout marshalling) and gathers
outputs; all FLOPs of the reference live on the device.
"""

import sys

if "/opt/trn_rl_repo" not in sys.path:
    sys.path.insert(0, "/opt/trn_rl_repo")

import numpy as np

B, L, D, C = 8, 2048, 512, 3
P = 128
NT = L // P          # 16 token tiles
NCH = L // 512       # 4 token chunks
NK = D // P          # 4 feature tiles

_CACHE = {}


def _build_nc():
    import os
    import concourse.bacc as bacc
    import concourse.mybir as mybir
    import concourse.tile as tile

    stage = int(os.environ.get("BK_STAGE", "9"))

    dt = mybir.dt
    f32, f16, u8 = dt.float32, dt.float16, dt.uint8
    AF = mybir.ActivationFunctionType
    OP = mybir.AluOpType

    nc = bacc.Bacc("TRN2", target_bir_lowering=False, debug=False, num_devices=8)

    # ---- DRAM inputs (per-core, host-marshalled layouts) ----
    # activations: [chunk, p, dtile, col] so each chunk is one contiguous DMA
    sft = nc.dram_tensor("sft", [NCH, P, NK, 512], f32, kind="ExternalInput")
    dft = nc.dram_tensor("dft", [NCH, P, NK, 512], f32, kind="ExternalInput")
    sbt = nc.dram_tensor("sbt", [NCH, P, NK, 512], f32, kind="ExternalInput")  # shifted +1 token
    sbn = nc.dram_tensor("sbn", [L, D], f32, kind="ExternalInput")
    dbn = nc.dram_tensor("dbn", [L, D], f32, kind="ExternalInput")
    wgf = nc.dram_tensor("wgf", [P, NK, 512], f32, kind="ExternalInput")
    wgb = nc.dram_tensor("wgb", [P, NK, 512], f32, kind="ExternalInput")
    bgfh = nc.dram_tensor("bgfh", [P, NK], f32, kind="ExternalInput")   # 0.5*bgf tiled
    bgbr = nc.dram_tensor("bgbr", [1, 512], f32, kind="ExternalInput")
    ua = nc.dram_tensor("ua", [P, C * NK, 512], f32, kind="ExternalInput")  # 0.25*U[:, :D, :D]
    uh = nc.dram_tensor("uh", [P, NK, C], f32, kind="ExternalInput")        # 0.5*U[:, :D, D]
    vrow = nc.dram_tensor("vrow", [1, C * 512], f32, kind="ExternalInput")  # 0.5*U[:, D, :D] flat
    sval = nc.dram_tensor("sval", [1, C], f32, kind="ExternalInput")        # U[:, D, D]
    wea = nc.dram_tensor("wea", [4, C, 512], f32, kind="ExternalInput")     # [We_i rows 0-2; be_i]
    web = nc.dram_tensor("web", [3, C, 512], f32, kind="ExternalInput")     # We_i rows 3-5
    maskbig = nc.dram_tensor("maskbig", [P, NT], f32, kind="ExternalInput")  # 4096*mask (X-layout)
    invmask = nc.dram_tensor("invmask", [P, NT], f32, kind="ExternalInput")  # 1-mask (X-layout)

    # ---- DRAM outputs ----
    masks_o = nc.dram_tensor("masks_o", [L, L], u8, kind="ExternalOutput")
    aswt_o = nc.dram_tensor("aswt_o", [C, L + 1], f32, kind="ExternalOutput")
    al_o = [
        nc.dram_tensor(f"al{i}_o", [L, D], f32, kind="ExternalOutput") for i in range(3)
    ]

    # ---- NEFF-embedded constants ----
    id_np = np.eye(P, dtype=np.float32)
    su_np = np.triu(np.ones((P, P), np.float32), 1)  # [q, p] = 1 iff q < p
    on_np = np.ones((P, P), np.float32)
    id_dr = nc.inline_tensor(id_np, "id128")
    su_dr = nc.inline_tensor(su_np, "su128")
    on_dr = nc.inline_tensor(on_np, "ones128")
    # row 0 = ones (DMA-source for bufA ones row)
    misc_np = np.zeros((1, L + 1), np.float32)
    misc_np[0] = 1.0
    misc_dr = nc.inline_tensor(misc_np, "misc4")

    with tile.TileContext(nc) as tc:
        with (
            tc.tile_pool(name="per", bufs=1) as per,
            tc.tile_pool(name="pin", bufs=2) as pin,
            tc.tile_pool(name="pbn", bufs=2) as pbn,
            tc.tile_pool(name="pth", bufs=2) as pth,
            tc.tile_pool(name="pm", bufs=1) as pm,
            tc.tile_pool(name="pbgs", bufs=2) as pbgs,
            tc.tile_pool(name="pscr", bufs=1) as pscr,
            tc.tile_pool(name="pal", bufs=2) as pal,
            tc.tile_pool(name="pmask", bufs=2) as pmask,
            tc.tile_pool(name="pasw", bufs=2) as pasw,
            tc.tile_pool(name="pp_pre", bufs=2, space="PSUM") as pp_pre,
            tc.tile_pool(name="pp_hh", bufs=3, space="PSUM") as pp_hh,
            tc.tile_pool(name="pp_up", bufs=2, space="PSUM") as pp_up,
            tc.tile_pool(name="pp_qq", bufs=1, space="PSUM") as pp_qq,
        ):
            # persistent SBUF
            wgf_sb = per.tile([P, NK, 512], f32, name="wgf_sb")
            wgb_sb = per.tile([P, NK, 512], f32, name="wgb_sb")
            bgfh_sb = per.tile([P, NK], f32, name="bgfh_sb")
            bgbr_sb = per.tile([1, 512], f32, name="bgbr_sb")
            ua_sb = per.tile([P, C * NK, 512], f32, name="ua_sb")
            uh_sb = per.tile([P, NK, C], f32, name="uh_sb")
            vrow_sb = per.tile([1, C * 512], f32, name="vrow_sb")
            sval_sb = per.tile([1, C], f32, name="sval_sb")
            wea_sb = per.tile([4, C, 512], f32, name="wea_sb")
            web_sb = per.tile([3, C, 512], f32, name="web_sb")
            maskbig_sb = per.tile([P, NT], f32, name="maskbig_sb")
            invmask_sb = per.tile([P, NT], f32, name="invmask_sb")
            id_sb = per.tile([P, P], f32, name="id_sb")
            su_sb = per.tile([P, P], f32, name="su_sb")
            ones_sb = per.tile([P, P], f32, name="ones_sb")
            fgs_sb = per.tile([P, NK, L], f32, name="fgs_sb")
            bufa = per.tile([4, L + 1], f32, name="bufa")
            x2 = per.tile([P, NT], f32, name="x2")
            qsb = per.tile([P, NT], f32, name="qsb")
            segm16 = per.tile([P, NT], f32, name="segm16")
            qt_sb = per.tile([NT, P], f32, name="qt_sb")
            qflat = per.tile([1, L], f32, name="qflat")
            qb16 = per.tile([P, L], f16, name="qb16")
            t_sb = per.tile([1, NT], f32, name="t_sb")
            tt_sb = per.tile([NT, 1], f32, name="tt_sb")
            carry_sb = per.tile([1, NT], f32, name="carry_sb")

            for t, dr in [
                (wgf_sb, wgf), (wgb_sb, wgb), (bgfh_sb, bgfh), (bgbr_sb, bgbr),
                (ua_sb, ua), (uh_sb, uh), (vrow_sb, vrow), (sval_sb, sval),
                (wea_sb, wea), (web_sb, web), (maskbig_sb, maskbig), (invmask_sb, invmask),
                (id_sb, id_dr), (su_sb, su_dr), (ones_sb, on_dr),
            ]:
                nc.sync.dma_start(t[:], dr.ap())

            nc.vector.memset(x2[:], 0.0)
            nc.vector.memset(bufa[0:3, :], 0.0)
            asw_tiles = []
            misc_sb = per.tile([1, L + 1], f32, name="misc_sb")
            nc.sync.dma_start(misc_sb[:], misc_dr.ap())
            nc.sync.dma_start(bufa[3:4, :], misc_sb[0:1, :])

            for ch in range(NCH):
                c0 = ch * 512
                sft_t = pin.tile([P, NK, 512], f32, name="sft_t", tag="sft")
                dft_t = pin.tile([P, NK, 512], f32, name="dft_t", tag="dft")
                sbt_t = pin.tile([P, NK, 512], f32, name="sbt_t", tag="sbt")
                nc.sync.dma_start(sft_t[:], sft.ap()[ch])
                nc.sync.dma_start(dft_t[:], dft.ap()[ch])
                nc.sync.dma_start(sbt_t[:], sbt.ap()[ch])

                # ---- f-side gating (transposed space) ----
                for j in range(NK):
                    pre = pp_pre.tile([P, 512], f32, name="pre", tag="pre")
                    for q in range(NK):
                        nc.tensor.matmul(
                            pre[:], wgf_sb[:, q, j * P : (j + 1) * P], sft_t[:, q, :],
                            start=(q == 0), stop=(q == NK - 1),
                        )
                    th = pth.tile([P, 512], f32, name="th", tag="th")
                    nc.scalar.activation(
                        th[:], pre[:], AF.Tanh, bias=bgfh_sb[:, j : j + 1], scale=0.5
                    )
                    m = pm.tile([P, 512], f32, name="m", tag="m")
                    nc.vector.scalar_tensor_tensor(
                        m[:], th[:], 0.0, dft_t[:, j, :], OP.bypass, OP.mult
                    )
                    nc.vector.scalar_tensor_tensor(
                        fgs_sb[:, j, c0 : c0 + 512], m[:], 0.0, sft_t[:, j, :],
                        OP.bypass, OP.add,
                    )

                if stage < 2:
                    continue
                for jj in range(4):
                    k = ch * 4 + jj
                    t0 = k * P
                    M = P - 1 if k == NT - 1 else P

                    # ---- b-side gating (natural space, shifted +1 token) ----
                    sb_t = pbn.tile([P, 512], f32, name="sb_t", tag="sbn")
                    db_t = pbn.tile([P, 512], f32, name="db_t", tag="dbn")
                    nc.sync.dma_start(sb_t[:M], sbn.ap()[t0 + 1 : t0 + 1 + M, :])
                    nc.sync.dma_start(db_t[:M], dbn.ap()[t0 + 1 : t0 + 1 + M, :])
                    preb = pp_pre.tile([P, 512], f32, name="preb", tag="pre")
                    for q in range(NK):
                        nc.tensor.matmul(
                            preb[:M], sbt_t[:, q, jj * P : jj * P + M], wgb_sb[:, q, :],
                            start=(q == 0), stop=False,
                        )
                    nc.tensor.matmul(
                        preb[:M], ones_sb[0:1, 0:M], bgbr_sb[:], start=False, stop=True
                    )
                    thb = pth.tile([P, 512], f32, name="thb", tag="th")
                    nc.scalar.activation(thb[:M], preb[:M], AF.Tanh, scale=0.5)
                    mb = pm.tile([P, 512], f32, name="mb", tag="m")
                    nc.vector.scalar_tensor_tensor(
                        mb[:M], thb[:M], 0.0, db_t[:M], OP.bypass, OP.mult
                    )
                    bgs = pbgs.tile([P, 512], f32, name="bgs", tag="bgs")
                    nc.vector.scalar_tensor_tensor(
                        bgs[:M], mb[:M], 0.0, sb_t[:M], OP.bypass, OP.add
                    )

                    if stage < 3:
                        continue
                    # ---- biaffine: H_c = fgs @ (U_c/4) + v_c/2 ; u-terms ----
                    hhs = []
                    for cc in range(C):
                        hh = pp_hh.tile([P, 512], f32, name="hh", tag="hh")
                        hhs.append(hh)
                        for i in range(NK):
                            nc.tensor.matmul(
                                hh[:M, :],
                                fgs_sb[:, i, t0 : t0 + M], ua_sb[:, cc * NK + i, :],
                                start=(i == 0), stop=False,
                            )
                        nc.tensor.matmul(
                            hh[:M, :],
                            ones_sb[0:1, 0:M], vrow_sb[0:1, cc * 512 : (cc + 1) * 512],
                            start=False, stop=True,
                        )
                    # u-term: [M, 3] = fgs @ (u_c/2) + s_c in one N=3 group
                    u_ps = pp_up.tile([P, 4], f32, name="u_ps", tag="up")
                    for i in range(NK):
                        nc.tensor.matmul(
                            u_ps[:M, 0:C],
                            fgs_sb[:, i, t0 : t0 + M], uh_sb[:, i, 0:C],
                            start=(i == 0), stop=False,
                        )
                    nc.tensor.matmul(
                        u_ps[:M, 0:C],
                        ones_sb[0:1, 0:M], sval_sb[0:1, 0:C],
                        start=False, stop=True,
                    )

                    asw_nat = pasw.tile([P, 4], f32, name="asw_nat", tag="asw", bufs=NT + 1)
                    acc3 = pasw.tile([P, 4], f32, name="acc3", tag="acc")
                    for cc in range(C):
                        scr = pscr.tile([P, 512], f32, name="scr", tag="scr")
                        nc.vector.scalar_tensor_tensor(
                            scr[:M], hhs[cc][:M, :], 0.0, bgs[:M],
                            OP.bypass, OP.mult, accum_out=acc3[:M, cc : cc + 1],
                        )
                    nc.vector.tensor_add(asw_nat[:M, 0:C], acc3[:M, 0:C], u_ps[:M, 0:C])
                    # is2 = asw2 > max(asw0, asw1)  (strict argmax==2)
                    nc.vector.tensor_max(
                        asw_nat[:M, 3:4], asw_nat[:M, 0:1], asw_nat[:M, 1:2]
                    )
                    nc.vector.tensor_tensor(
                        x2[:M, k : k + 1], asw_nat[:M, 2:3], asw_nat[:M, 3:4], OP.is_gt
                    )
                    asw_tiles.append((k, t0, M, asw_nat))

            # ---- deferred: asw transposes into bufa (PE runs dense) ----
            if stage >= 3:
                for (k, t0, M, asw_nat) in asw_tiles:
                    tp = pp_up.tile([NT, P], f32, name="tp", tag="up")
                    nc.tensor.transpose(
                        tp[0:3, 0:M], asw_nat[0:M, 0:3], id_sb[0:M, 0:M]
                    )
                    nc.scalar.activation(
                        bufa[0:3, t0 + 1 : t0 + 1 + M], tp[0:3, 0:M], AF.Copy
                    )
            # ---- deferred: al1/al2/al3 matmuls ----
            if stage >= 4:
                for (k, t0, M, _a) in asw_tiles:
                    for i in range(3):
                        alp = pp_hh.tile([P, 512], f32, name="alp", tag="hh")
                        nc.tensor.matmul(
                            alp[:], bufa[0:4, t0 + 1 : t0 + 1 + P], wea_sb[:, i, :],
                            start=True, stop=False,
                        )
                        nc.tensor.matmul(
                            alp[:], bufa[0:3, t0 : t0 + P], web_sb[:, i, :],
                            start=False, stop=True,
                        )
                        al_sb = pal.tile([P, 512], f32, name="al_sb", tag="al")
                        nc.scalar.activation(
                            al_sb[:], alp[:], AF.Copy, scale=invmask_sb[:, k : k + 1]
                        )
                        nc.sync.dma_start(al_o[i].ap()[t0 : t0 + P, :], al_sb[:])

            if stage < 2:
                nc.sync.dma_start(al_o[0].ap()[0:P, :], fgs_sb[:, 0, 0:512])
            if stage >= 3:
                nc.sync.dma_start(aswt_o.ap()[:, :], bufa[0:3, :])
            if stage >= 5:
                # ---- strict prefix sum Q (two-level) ----
                # within-chunk strict cumsum: Y = SU^T X
                q_ps = pp_qq.tile([P, NT], f32, name="q_ps", tag="qq")
                nc.tensor.matmul(
                    q_ps[:, 0:NT], su_sb[:], x2[:, 0:NT],
                    start=True, stop=False, skip_group_check=True,
                )
                # chunk sums T[k] = colsum(X[:, k])
                t_ps = pp_up.tile([1, NT], f32, name="t_ps", tag="up")
                nc.tensor.matmul(
                    t_ps[0:1, 0:NT], ones_sb[:, 0:1], x2[:, 0:NT],
                    start=True, stop=True, skip_group_check=True,
                )
                nc.vector.tensor_copy(t_sb[:], t_ps[0:1, :])
                # transpose T to [NT, 1]
                tt_ps = pp_up.tile([NT, 1], f32, name="tt_ps", tag="up")
                nc.tensor.transpose(tt_ps[0:NT, 0:1], t_sb[0:1, 0:NT], id_sb[0:1, 0:1])
                nc.vector.tensor_copy(tt_sb[:], tt_ps[0:NT, :])
                # strict cumsum of chunk sums: carry[k] = sum_{q<k} T[q]
                carry_ps = pp_up.tile([1, NT], f32, name="carry_ps", tag="up")
                nc.tensor.matmul(
                    carry_ps[0:1, 0:NT], tt_sb[0:NT, 0:1], su_sb[0:NT, 0:NT],
                    start=True, stop=True, skip_group_check=True,
                )
                nc.vector.tensor_copy(carry_sb[:], carry_ps[0:1, :])
                # broadcast-accumulate carry into Q
                nc.tensor.matmul(
                    q_ps[:, 0:NT], ones_sb[0:1, 0:P], carry_sb[0:1, 0:NT],
                    start=False, stop=True, skip_group_check=True,
                )
                nc.vector.tensor_copy(qsb[:], q_ps[:])
                # row scalars with row-mask folded in: Q - 4096*mask
                nc.vector.scalar_tensor_tensor(
                    segm16[:], qsb[:], 0.0, maskbig_sb[:], OP.bypass, OP.subtract
                )
                # flatten Q to [1, L]: PE transpose + sbuf->sbuf DMA
                qt_ps = pp_up.tile([NT, P], f32, name="qt_ps", tag="up")
                nc.tensor.transpose(qt_ps[0:NT, 0:P], qsb[:, 0:NT], id_sb[:, :])
                nc.scalar.activation(qt_sb[:], qt_ps[0:NT, :], AF.Copy)
                nc.sync.dma_start(qflat[0:1, :], qt_sb[:])
                # broadcast Q over partitions (fp16)
                for cch in range(NCH):
                    bp = pp_up.tile([P, 512], f32, name="bp", tag="up")
                    nc.tensor.matmul(
                        bp[:], ones_sb[0:1, 0:P], qflat[0:1, cch * 512 : (cch + 1) * 512],
                        start=True, stop=True,
                    )
                    nc.vector.tensor_copy(qb16[:, cch * 512 : (cch + 1) * 512], bp[:])
                # L x L equality mask
                for k in range(NT):
                    mt = pmask.tile([P, L], u8, name="mt", tag="mask")
                    nc.vector.tensor_scalar(
                        mt[:], qb16[:], segm16[:, k : k + 1], None, OP.is_equal
                    )
                    nc.sync.dma_start(masks_o.ap()[k * P : (k + 1) * P, :], mt[:])

    nc.compile()
    return nc


def _prep_core_inputs(bi, f, c, b, mask, Wgf, bgf, Wgb, bgb, U, WE):
    fb, cb, bb = f[bi], c[bi], b[bi]
    s_f = fb + cb
    d_f = fb - cb
    s_b = bb + cb
    d_b = bb - cb

    def chunks_T(x, shift):
        # x: [L, D] -> transposed chunked layout [NCH, P, NK, 512] over tokens,
        # token columns optionally shifted by +1 (padded with zero)
        xt = x.T  # [D, L]
        if shift:
            xt = np.concatenate([xt[:, 1:], np.zeros((D, 1), np.float32)], axis=1)
        out = np.empty((NCH, P, NK, 512), np.float32)
        for ch in range(NCH):
            blk = xt[:, ch * 512 : (ch + 1) * 512]  # [D, 512]
            out[ch] = blk.reshape(NK, P, 512).transpose(1, 0, 2)
        return out

    mk = mask[bi, 0].astype(np.float32)  # [L]
    mk_x = mk.reshape(NT, P).T.copy()    # [P, NT]

    return {
        "sft": chunks_T(s_f, False),
        "dft": chunks_T(d_f, False),
        "sbt": chunks_T(s_b, True),
        "sbn": np.ascontiguousarray(s_b),
        "dbn": np.ascontiguousarray(d_b),
        "wgf": Wgf.reshape(NK, P, 512).transpose(1, 0, 2).copy(),
        "wgb": Wgb.reshape(NK, P, 512).transpose(1, 0, 2).copy(),
        "bgfh": (0.5 * bgf).reshape(NK, P).T.copy(),
        "bgbr": bgb.reshape(1, 512).copy(),
        "ua": (0.25 * U[:, :D, :D]).reshape(C, NK, P, 512).transpose(2, 0, 1, 3).reshape(P, C * NK, 512).copy(),
        "uh": (0.5 * U[:, :D, D]).T.reshape(NK, P, C).transpose(1, 0, 2).copy(),
        "vrow": (0.5 * U[:, D, :D]).reshape(1, C * 512).copy(),
        "sval": U[:, D, D].reshape(1, C).copy(),
        "wea": WE[0],
        "web": WE[1],
        "maskbig": 4096.0 * mk_x,
        "invmask": 1.0 - mk_x,
    }


def _pack_we(We1, be1, We2, be2, We3, be3):
    # al1 <- (We1, be1); al2 <- (We3, be3); al3 <- (We2, be2)  (reference quirk)
    # wea = [We rows 0-2; be], web = We rows 3-5
    pairs = [(We1, be1), (We3, be3), (We2, be2)]
    wea = np.stack(
        [
            np.concatenate([np.asarray(w, np.float32)[0:3], np.asarray(bias, np.float32)[None, :]], 0)
            for w, bias in pairs
        ],
        axis=1,
    ).copy()  # [4, 3, 512]
    web = np.stack(
        [np.asarray(w, np.float32)[3:6] for w, _ in pairs], axis=1
    ).copy()  # [3, 3, 512]
    return (wea, web)


def kernel(f, c, b, mask, Wgf, bgf, Wgb, bgb, U, We1, be1, We2, be2, We3, be3):
    from concourse import bass_utils

    if "nc" not in _CACHE:
        _CACHE["nc"] = _build_nc()
    nc = _CACHE["nc"]

    f = np.asarray(f, np.float32)
    c = np.asarray(c, np.float32)
    b = np.asarray(b, np.float32)
    mask = np.asarray(mask)
    U = np.asarray(U, np.float32)
    WE = _pack_we(We1, be1, We2, be2, We3, be3)

    in_maps = [
        _prep_core_inputs(
            bi, f, c, b, mask,
            np.asarray(Wgf, np.float32), np.asarray(bgf, np.float32),
            np.asarray(Wgb, np.float32), np.asarray(bgb, np.float32),
            U, WE,
        )
        for bi in range(B)
    ]

    r = bass_utils.run_bass_kernel_spmd(nc, in_maps, core_ids=list(range(B)))

    asw = np.stack([r.results[bi]["aswt_o"][:, 1:L].T for bi in range(B)])
    masks = np.stack([r.results[bi]["masks_o"].astype(bool) for bi in range(B)])
    al1 = np.stack([r.results[bi]["al0_o"] for bi in range(B)])
    al2 = np.stack([r.results[bi]["al1_o"] for bi in range(B)])
    al3 = np.stack([r.results[bi]["al2_o"] for bi in range(B)])
    return asw, masks, al1, al2, al3
